# revision 1
# baseline (speedup 1.0000x reference)
"""YOLOv1 loss kernel for Trainium2 (8 NeuronCores, data-parallel over batch).

Strategy: the loss splits exactly into
    total = sum_{obj cells} (coor_sel + e_sel^2 + 0.5*e_oth^2 + cls)
          + sum_{noobj cells} 0.5*(p4^2 + p9^2)
where obj = (labels[:,4] == 1.0).  Only ~30% of cells are obj.  The host
packer (a permutation + fp16 cast + constant channel scaling) partitions
cells by the obj flag per core:
  - obj region: 32 channels/cell, full IoU/coor/conf/cls pipeline.
  - noobj region: only pred conf channels (2/cell); the whole
    contribution is ONE ACT Square(scale sqrt(.5)) with accum_out.
This cuts DMA from 6.6MB to ~2.2MB/core and elementwise work ~70%.

Channel blocks (cells along columns, 128 partitions; within 4W blocks
the order is [*_box1, *_box2] per axis so x/y pair via W-strided views):
  PQs = 3.5*pred [w1,w2,h1,h2]  (ch 2,7,3,8)   cols  0: 4W
  LSQ = 3.5*lab  [wg,l7,hg,l8]  (ch 2,7,3,8)   cols  4: 8W
  LXY = lab  [xg,l5,yg,l6]      (ch 0,5,1,6)   cols  8:12W
  PA  = pred [x1,x2,y1,y2]      (ch 0,5,1,6)   cols 12:16W
  PF  = pred [c1,c2]            (ch 4,9)       cols 16:18W
  PC  = pred cls                (ch 10..16)    cols 18:25W
  LC  = lab  cls                (ch 10..16)    cols 25:32W
The 3.5 pre-scale makes PQs/LSQ the IoU half-widths directly; the coor
sqrt terms absorb it via the sds ACT scale sqrt(10/7) ((sqrt(3.5p) -
sqrt(3.5l))^2 = 3.5*(sqrt p - sqrt l)^2).  The ground box (xg,yg,wg,hg)
doubles as the coor1 target.

The merge tail is decomposed so no per-cell o3 is ever formed:
  sum o3 = sum coorp2 + sum sa + sum clsf + alpha + 2*beta
with sa = u1*(dcoor+des), alpha = sum es1, beta = sum es2 - the es/cp
sums ride ACT accum_out registers, cutting the serial DVE tail.

Padding cells (to equalize the 8 cores' shapes) use identical pred/label
boxes with conf=1: contribution is only LUT roundoff (~1e-5 each).

IoU: translation invariance drops the grid offsets; with coords x7 the
box is center=x, half=3.5w, and inter/areas share a 1/4 factor that
cancels in inter/union.

ACT tables: sqrt and rsqrt never share a set, but square is in every
set.  Order: warm Sqrt (loads during DMA fill), spl, then a dummy warm
Rsqrt triggers the single switch ~3us in (hidden under DVE box math);
all later squares and the real Rsqrt run from the rsqrt set.
"""

import numpy as np

B = 16384
NCORES = 8
BL = B // NCORES
CELLS = 49
NFLAT = BL * CELLS        # 100352 cells per core
P = 128

SQRT5 = float(np.sqrt(5.0))
SQRTH = float(np.sqrt(0.5))
SDS_SCALE = float(np.sqrt(10.0 / 7.0))

# channel gather orders (index into the 17 channels)
_PRED_WH = [2, 7, 3, 8]
_LAB_WH = [2, 7, 3, 8]
_PRED_XY = [0, 5, 1, 6]
_LAB_XY = [0, 5, 1, 6]
_CLS = [10, 11, 12, 13, 14, 15, 16]

# pad cell: identical boxes (0.5 everywhere), conf 1.0 -> contribution ~0
_PAD = np.zeros(32, np.float16)
_PAD[0:4] = 1.75          # PQs (3.5 * 0.5)
_PAD[4:8] = 0.5           # PA
_PAD[8:12] = 1.75         # LSQ
_PAD[12:16] = 0.5         # LXY
_PAD[16:18] = 1.0         # PF
_PAD[18:32] = 0.5         # PC, LC


def _pack_all(pred, labels):
    """-> (xo (NC,P,32*WO) f16, xn (NC,P,2*WN) f16, WO, WN)"""
    prd = np.ascontiguousarray(
        np.asarray(pred, np.float32).reshape(NCORES, BL, 17, CELLS)
        .transpose(0, 2, 1, 3)).reshape(NCORES, 17, NFLAT)
    lab = np.ascontiguousarray(
        np.asarray(labels, np.float32).reshape(NCORES, BL, 17, CELLS)
        .transpose(0, 2, 1, 3)).reshape(NCORES, 17, NFLAT)
    objf = lab[:, 4, :] == 1.0
    counts = objf.sum(1)
    WO = max(1, -(-int(counts.max()) // P))
    WN = max(1, -(-int(NFLAT - counts.min()) // P))
    NO = P * WO
    NN = P * WN

    xo = np.empty((NCORES, 32, NO), np.float16)
    xn = np.zeros((NCORES, 2, NN), np.float16)
    for i in range(NCORES):
        oi = np.flatnonzero(objf[i])
        ni = np.flatnonzero(~objf[i])
        c = len(oi)
        xo[i, 0:4, :c] = 3.5 * prd[i][_PRED_WH][:, oi]
        xo[i, 4:8, :c] = prd[i][_PRED_XY][:, oi]
        xo[i, 8:12, :c] = 3.5 * lab[i][_LAB_WH][:, oi]
        xo[i, 12:16, :c] = lab[i][_LAB_XY][:, oi]
        xo[i, 16:18, :c] = prd[i][[4, 9]][:, oi]
        xo[i, 18:25, :c] = prd[i][_CLS][:, oi]
        xo[i, 25:32, :c] = lab[i][_CLS][:, oi]
        xo[i, :, c:] = _PAD[:, None]
        xn[i, :, :len(ni)] = prd[i][[4, 9]][:, ni]
    # (NC,32,NO) -> (NC,P,32,WO) -> (NC,P,32*WO); cell k = p*WO + j
    xo = xo.reshape(NCORES, 32, P, WO).transpose(0, 2, 1, 3)
    xo = np.ascontiguousarray(xo).reshape(NCORES, P, 32 * WO)
    xn = xn.reshape(NCORES, 2, P, WN).transpose(0, 2, 1, 3)
    xn = np.ascontiguousarray(xn).reshape(NCORES, P, 2 * WN)
    return xo, xn, WO, WN


def _act_rsqrt(nc, mybir, out, in_):
    """ScalarE Rsqrt via raw InstActivation (bass wrapper bans Rsqrt);
    1/union = rsqrt(union)^2, measured end-to-end error ~1e-5."""
    imm = lambda v: mybir.ImmediateValue(dtype=mybir.dt.float32, value=v)
    eng = nc.scalar
    inst = mybir.InstActivation(
        name=nc.get_next_instruction_name(),
        func=mybir.ActivationFunctionType.Rsqrt,
        ins=[eng.lower_ap(in_), imm(0.0), imm(1.0), imm(0.0)],
        outs=[eng.lower_ap(out)],
    )
    return eng.add_instruction(inst)


def _build_nc(WO, WN):
    import concourse.bass as bass
    import concourse.mybir as mybir
    from concourse.tile import TileContext
    from concourse.alu_op_type import AluOpType as op

    CT = mybir.dt.float16
    F32 = mybir.dt.float32
    SQ = mybir.ActivationFunctionType.Square
    SQRT = mybir.ActivationFunctionType.Sqrt
    W = WO

    nc = bass.Bass()
    xo_in = nc.dram_tensor("xo", [P, 32 * W], CT, kind="ExternalInput")
    xn_in = nc.dram_tensor("xn", [P, 2 * WN], CT, kind="ExternalInput")
    acc_out = nc.dram_tensor("acc", [P, 6], F32, kind="ExternalOutput")

    def v22(ap):   # [P,4W] -> [P,2,2,W] (axis, box, w)
        return ap.rearrange("p (a o w) -> p a o w", a=2, o=2)

    def v21(ap):   # [P,2W] -> [P,2,1,W]
        return ap.rearrange("p (a o w) -> p a o w", a=2, o=1)

    def bco(ap):   # [P,2,1,W] -> [P,2,2,W]
        return ap.broadcast_to([P, 2, 2, W])

    with TileContext(nc) as tc:
        with (
            tc.tile_pool(name="inp", bufs=1) as inpool,
            tc.tile_pool(name="mid", bufs=1) as mid,
            tc.tile_pool(name="accp", bufs=1) as accp,
        ):
            acc = accp.tile([P, 6], F32)

            # DMA pieces: per-queue FIFO follows descgen order, so
            # submission order sets cumulative completion; box inputs first.
            # The scalar dma dispatches precede the table-load warm-up so
            # descgen is not stuck behind the 1.28us load.
            xot = inpool.tile([P, 32 * W], CT)
            xnt = inpool.tile([P, 2 * WN], CT)
            nc.sync.dma_start(out=xot[:, 0:8 * W], in_=xo_in[:, 0:8 * W])
            nc.scalar.dma_start(out=xot[:, 8 * W:16 * W],
                                in_=xo_in[:, 8 * W:16 * W])
            nc.scalar.dma_start(out=xot[:, 16 * W:32 * W],
                                in_=xo_in[:, 16 * W:32 * W])
            nc.gpsimd.dma_start(out=xnt[:], in_=xn_in[:])

            warm = accp.tile([P, 2], CT)
            nc.vector.memset(warm[:], 1.0)
            wo_ = accp.tile([P, 2], CT)
            # loads the sqrt table set while the DMAs are in flight
            nc.scalar.activation(out=wo_[:], in_=warm[:], func=SQRT)

            PQs = xot[:, 0:4 * W]            # 3.5*[w1,w2,h1,h2]
            PA = xot[:, 4 * W:8 * W]         # [x1,x2,y1,y2]
            LSQ = xot[:, 8 * W:12 * W]       # 3.5*[wg,l7,hg,l8]
            LXY = xot[:, 12 * W:16 * W]      # [xg,l5,yg,l6]
            PF = xot[:, 16 * W:18 * W]       # [c1,c2]
            PC = xot[:, 18 * W:25 * W]
            LC = xot[:, 25 * W:32 * W]
            LSQg = v22(LSQ)[:, :, 0:1]       # [P,2,1,W] = 3.5*[wg,hg]
            LXYg = v22(LXY)[:, :, 0:1]       # [P,2,1,W] = [xg,yg]

            # ACT: sqrt of pred then label wh blocks; dummy rsqrt (reads
            # sqg so it stays after the last Sqrt) pulls the single table
            # switch early, hidden under DVE box math
            sqp = mid.tile([P, 4 * W], CT)
            nc.scalar.activation(out=sqp[:], in_=PQs, func=SQRT)
            sqg = mid.tile([P, 4 * W], CT)
            nc.scalar.activation(out=sqg[:], in_=LSQ, func=SQRT)
            _act_rsqrt(nc, mybir, wo_[:], sqg[:, 0:2])

            # --- DVE stream (queue order ~= execution order) ---
            arp = mid.tile([P, 2 * W], CT)
            nc.vector.tensor_tensor(out=arp[:], in0=PQs[:, 0:2 * W],
                                    in1=PQs[:, 2 * W:4 * W], op=op.mult)
            x1p = mid.tile([P, 4 * W], CT)
            nc.vector.tensor_tensor(out=x1p[:], in0=PA, in1=PQs, op=op.subtract)
            x2p = mid.tile([P, 4 * W], CT)
            nc.vector.tensor_tensor(out=x2p[:], in0=PA, in1=PQs, op=op.add)
            arg2 = mid.tile([P, 2 * W], CT)
            nc.vector.tensor_tensor(out=arg2[:, 0:W], in0=LSQ[:, 0:W],
                                    in1=LSQ[:, 2 * W:3 * W], op=op.mult)
            nc.vector.tensor_tensor(out=arg2[:, W:2 * W], in0=LSQ[:, 0:W],
                                    in1=LSQ[:, 2 * W:3 * W], op=op.mult)
            x1g = mid.tile([P, 2 * W], CT)
            nc.vector.tensor_tensor(out=v21(x1g[:]), in0=LXYg, in1=LSQg,
                                    op=op.subtract)
            x2g = mid.tile([P, 2 * W], CT)
            nc.vector.tensor_tensor(out=v21(x2g[:]), in0=LXYg, in1=LSQg,
                                    op=op.add)
            imax = mid.tile([P, 4 * W], CT)
            nc.vector.tensor_tensor(out=v22(imax[:]), in0=v22(x1p[:]),
                                    in1=bco(v21(x1g[:])), op=op.max)
            imin = mid.tile([P, 4 * W], CT)
            nc.vector.tensor_tensor(out=v22(imin[:]), in0=v22(x2p[:]),
                                    in1=bco(v21(x2g[:])), op=op.min)
            dd = mid.tile([P, 4 * W], CT)
            nc.vector.tensor_tensor(out=dd[:], in0=imin[:], in1=imax[:],
                                    op=op.subtract)
            dr = mid.tile([P, 4 * W], CT)
            nc.vector.tensor_scalar(out=dr[:], in0=dd[:], scalar1=0.0,
                                    scalar2=0.5, op0=op.max, op1=op.mult)
            inter = mid.tile([P, 2 * W], CT)
            nc.vector.tensor_tensor(out=inter[:], in0=dr[:, 0:2 * W],
                                    in1=dr[:, 2 * W:4 * W], op=op.mult)
            uu = mid.tile([P, 2 * W], CT)
            nc.vector.tensor_tensor(out=uu[:], in0=arp[:], in1=arg2[:],
                                    op=op.add)
            un = mid.tile([P, 2 * W], CT)
            nc.vector.tensor_tensor(out=un[:], in0=uu[:], in1=inter[:],
                                    op=op.subtract)

            # ACT: rc fires the moment un lands; squares follow
            rc = mid.tile([P, 2 * W], CT)
            _act_rsqrt(nc, mybir, rc[:], un[:])
            sd = mid.tile([P, 4 * W], CT)
            nc.vector.tensor_tensor(out=sd[:], in0=sqp[:], in1=sqg[:],
                                    op=op.subtract)
            diffa = mid.tile([P, 4 * W], CT)
            nc.vector.tensor_tensor(out=diffa[:], in0=PA, in1=LXY,
                                    op=op.subtract)
            dsqa = mid.tile([P, 4 * W], CT)
            nc.scalar.activation(out=dsqa[:], in_=diffa[:], func=SQ, scale=SQRT5)
            sds = mid.tile([P, 4 * W], CT)
            nc.scalar.activation(out=sds[:], in_=sd[:], func=SQ, scale=SDS_SCALE)

            ih = mid.tile([P, 2 * W], CT)
            nc.vector.tensor_tensor(out=ih[:], in0=inter[:], in1=rc[:],
                                    op=op.mult)
            iou = mid.tile([P, 2 * W], CT)
            nc.vector.tensor_tensor(out=iou[:], in0=ih[:], in1=rc[:],
                                    op=op.mult)
            u1c = mid.tile([P, W], CT)
            nc.vector.tensor_tensor(out=u1c[:], in0=iou[:, 0:W],
                                    in1=iou[:, W:2 * W], op=op.is_ge)
            e = mid.tile([P, 2 * W], CT)
            nc.vector.tensor_tensor(out=e[:], in0=PF, in1=iou[:], op=op.subtract)

            # ACT: es halves accumulate alpha = sum es1, beta = sum es2
            ce = mid.tile([P, 4 * W], CT)       # [coorp(2W) | es(2W)]
            es = ce[:, 2 * W:4 * W]
            nc.scalar.activation(out=ce[:, 2 * W:3 * W], in_=e[:, 0:W], func=SQ,
                                 scale=SQRTH, accum_out=acc[:, 0:1])
            nc.scalar.activation(out=ce[:, 3 * W:4 * W], in_=e[:, W:2 * W],
                                 func=SQ, scale=SQRTH, accum_out=acc[:, 1:2])

            diffc = mid.tile([P, 7 * W], CT)
            nc.vector.tensor_tensor(out=diffc[:], in0=PC, in1=LC,
                                    op=op.subtract)
            dsqc = mid.tile([P, 7 * W], CT)
            nc.vector.tensor_tensor(out=dsqc[:], in0=diffc[:], in1=diffc[:],
                                    op=op.mult)
            c1 = mid.tile([P, 3 * W], CT)
            nc.vector.tensor_tensor(out=c1[:], in0=dsqc[:, 0:3 * W],
                                    in1=dsqc[:, 3 * W:6 * W], op=op.add)
            c2 = mid.tile([P, W], CT)
            nc.vector.tensor_tensor(out=c2[:], in0=c1[:, 0:W],
                                    in1=c1[:, W:2 * W], op=op.add)
            c3 = mid.tile([P, W], CT)
            nc.vector.tensor_tensor(out=c3[:], in0=c2[:], in1=c1[:, 2 * W:3 * W],
                                    op=op.add)
            clsf = mid.tile([P, W], CT)
            nc.vector.tensor_tensor(out=clsf[:], in0=c3[:],
                                    in1=dsqc[:, 6 * W:7 * W], op=op.add)
            tq = mid.tile([P, 4 * W], CT)
            nc.vector.tensor_tensor(out=tq[:], in0=dsqa[:], in1=sds[:], op=op.add)
            coorp = ce[:, 0:2 * W]
            nc.vector.tensor_tensor(out=coorp, in0=tq[:, 0:2 * W],
                                    in1=tq[:, 2 * W:4 * W], op=op.add)
            # one strided op: [dcoor|des] = ce cols {0,2W} - cols {W,3W}
            dde = mid.tile([P, 2 * W], CT)
            cev = ce[:].rearrange("p (a b w) -> p a b w", a=2, b=2)
            nc.vector.tensor_tensor(out=dde[:].rearrange("p (a w) -> p a w", a=2),
                                    in0=cev[:, :, 0], in1=cev[:, :, 1],
                                    op=op.subtract)
            da = mid.tile([P, W], CT)
            nc.vector.tensor_tensor(out=da[:], in0=dde[:, 0:W],
                                    in1=dde[:, W:2 * W], op=op.add)
            sa = mid.tile([P, W], CT)
            nc.vector.tensor_tensor(out=sa[:], in0=u1c[:], in1=da[:], op=op.mult)
            nc.vector.tensor_reduce(out=acc[:, 2:3], in_=sa[:],
                                    axis=mybir.AxisListType.X, op=op.add)

            # ACT tail: noobj square-accum + off-path sums
            ppsn = mid.tile([P, 2 * WN], CT)
            nc.scalar.activation(out=ppsn[:], in_=xnt[:], func=SQ, scale=SQRTH,
                                 accum_out=acc[:, 5:6])
            cpc = mid.tile([P, W], CT)
            nc.scalar.activation(out=cpc[:], in_=clsf[:],
                                 func=mybir.ActivationFunctionType.Copy,
                                 accum_out=acc[:, 4:5])
            cps = mid.tile([P, W], CT)
            nc.scalar.activation(out=cps[:], in_=coorp[:, W:2 * W],
                                 func=mybir.ActivationFunctionType.Copy,
                                 accum_out=acc[:, 3:4])

            nc.sync.dma_start(out=acc_out[:], in_=acc[:])

    _split_multiwaits(nc, mybir)
    return nc


def _split_multiwaits(nc, mybir, max_waits=1):
    """This walrus build rejects instructions carrying more than one sem
    wait; hoist extra waits onto same-engine Drain instructions inserted
    immediately before the offender (semantically identical stall point)."""
    ctr = [0]
    for bb in nc.main_func.blocks:
        insts = bb.instructions
        out = []
        for ins in insts:
            si = ins.sync_info
            if si is not None and si.on_wait and len(si.on_wait) > max_waits:
                waits = list(si.on_wait)
                extra, keep = waits[:-max_waits], waits[-max_waits:]
                for k in range(0, len(extra), max_waits):
                    d = mybir.InstDrain(name=f"I-mw{ctr[0]}", ins=[], outs=[])
                    ctr[0] += 1
                    d.engine = ins.engine
                    d.sync_info = mybir.SyncInfo(on_wait=extra[k:k + max_waits],
                                                 on_update=[])
                    nc.register_instruction(d)
                    out.append(d)
                ins.sync_info = mybir.SyncInfo(on_wait=keep,
                                               on_update=list(si.on_update or []))
            out.append(ins)
        bb.instructions = out


_CACHED = {}


def kernel(pred, labels):
    from concourse.bass_utils import run_bass_kernel_spmd

    xo, xn, WO, WN = _pack_all(pred, labels)
    key = (WO, WN)
    if key not in _CACHED:
        _CACHED.clear()
        _CACHED[key] = _build_nc(WO, WN)
    nc = _CACHED[key]

    in_maps = [{"xo": xo[i], "xn": xn[i]} for i in range(NCORES)]
    res = run_bass_kernel_spmd(nc, in_maps, core_ids=list(range(NCORES)))
    wts = np.array([1.0, 2.0, 1.0, 1.0, 1.0, 1.0], np.float64)
    total = np.float64(0.0)
    for i in range(NCORES):
        a = res.results[i]["acc"].astype(np.float64)
        total += (a.sum(axis=0) * wts).sum()
    return np.asarray(total / B, dtype=np.float32)

